# revision 2
# baseline (speedup 1.0000x reference)
"""CrossModalAttentionScorer Trainium2 kernel (Bass/Tile, 8 NeuronCores), v3.

Reference computation (per batch b):
    R = anchor @ W_region            [A, H]
    Q = query  @ W_query             [T, H]
    S = R @ Q.T  (masked over T)     [A, T]
    P = softmax(S, axis=T)
    att = P @ Q                      [A, H]
    out = relu(concat([anchor, att, anchor*att]) @ W_combine + b)   [A, H]

Shapes: B=32, A=1024, T=64, D=H=512. Data-parallel over B across 8 cores.

v3 on top of v2's fusions (S = anchor @ (WrWq^T) @ q^T so R is never
materialized; att@Wc2 = P@(Q@(WqWc2)); bias folded via P rows summing to 1;
fp16 operands, fp32 psum):

  * Scores are computed TRANSPOSED: S^T [T, ACH] in 4 N=512 matmuls per
    chunk instead of 20 N=64 ones, and the attention-weight transposes
    disappear entirely (S^T IS the layout the attended/Y2 matmuls consume).
  * Constant-shift softmax: exp(s - 40 + mask) with the mask (0|-30000) and
    shift folded into the ACT bias (per-partition now, since T is the
    partition dim). No per-row max pass: logits are N(0,~22.6) so
    exp(s-40) stays within f32/bf16 range with astronomic margin (sum
    underflow needs all valid logits < -63, P ~ 1e-70 even for 1-token
    rows; overflow needs a logit > 128, ~5.7 sigma of the 2M-sample max).
  * Row sums via a ones-vector matmul on the idle K=64 PE path, reciprocal
    on DVE, partition-broadcast on the otherwise-idle GpSimd engine, one
    DVE multiply to produce normalized attn^T in fp16.
  * anchor (x) att product multiplies the attended psum directly on DVE
    (no ACT staging copy), halving ACT load.
"""
import numpy as np

import concourse.bacc as bacc
import concourse.tile as tile
import concourse.mybir as mybir
from concourse.bass_utils import run_bass_kernel_spmd

B, A, T, D, H = 32, 1024, 64, 512, 512
NCORES = 8
PB = B // NCORES          # batches per core = 4
P = 128                   # partitions
DT = D // P               # 4 d-tiles
HT = H // P               # 4 h-tiles
ACH = 512                 # a-chunk (moving-dim) size
NCH = A // ACH            # 2 chunks per batch
AT_CH = ACH // P          # 4 a-tiles per chunk
QTW = PB * T              # packed q width = 256
SHIFT = 40.0              # constant softmax shift (see module docstring)

F32 = mybir.dt.float32
F16 = mybir.dt.float16
BF16 = mybir.dt.bfloat16
AFT = mybir.ActivationFunctionType

_CACHE = {}


def build(reps: int = 1):
    """Build the per-core Bass module (4 batches of the problem).

    reps>1 repeats the whole computation in one NEFF for timing-by-slope."""
    nc = bacc.Bacc(None, target_bir_lowering=False, debug=False)

    aT = nc.dram_tensor("aT", [PB, D, A], F16, kind="ExternalInput")
    qT = nc.dram_tensor("qT", [D, QTW], F16, kind="ExternalInput")
    mbT = nc.dram_tensor("mbT", [PB, T, 1], F32, kind="ExternalInput")
    wq = nc.dram_tensor("wq", [D, H], F16, kind="ExternalInput")
    mqr = nc.dram_tensor("mqr", [D, D], F16, kind="ExternalInput")
    wqc2 = nc.dram_tensor("wqc2", [D, H], F16, kind="ExternalInput")
    wc1 = nc.dram_tensor("wc1", [D, H], F16, kind="ExternalInput")
    wc3 = nc.dram_tensor("wc3", [H, H], F16, kind="ExternalInput")
    bc = nc.dram_tensor("bc", [1, H], F16, kind="ExternalInput")
    x = nc.dram_tensor("x", [PB, A, H], F16, kind="ExternalOutput")

    with tile.TileContext(nc) as tc:
        with (
            tc.tile_pool(name="const", bufs=1) as const,
            tc.tile_pool(name="perrep", bufs=2) as perrep,
            tc.tile_pool(name="perb", bufs=2) as perb,
            tc.tile_pool(name="chunk", bufs=2) as chunk,
            tc.tile_pool(name="small", bufs=4) as small,
            tc.tile_pool(name="stage", bufs=3) as stage,
            tc.tile_pool(name="psum", bufs=4, space="PSUM") as psum,
        ):
            # ---- constants (only mqr loads up front; the rest are deferred
            # into the first chunks so the aT/qT streams start immediately) ----
            mqr_sb, wq_sb, wqc2_sb, wc1_sb, wc3_sb = [], [], [], [], []
            for d in range(DT):
                t = const.tile([P, D], F16, name=f"mqr{d}")
                nc.sync.dma_start(out=t, in_=mqr[d * P:(d + 1) * P, :])
                mqr_sb.append(t)
            for d in range(DT):
                wq_sb.append(const.tile([P, H], F16, name=f"wq{d}"))
                wqc2_sb.append(const.tile([P, H], F16, name=f"wqc2{d}"))
                wc1_sb.append(const.tile([P, H], F16, name=f"wc1{d}"))
                wc3_sb.append(const.tile([P, H], F16, name=f"wc3{d}"))
            bc_sb = const.tile([1, H], F16, name="bc_sb")
            nc.scalar.dma_start(out=bc_sb, in_=bc[:, :])
            ones16 = const.tile([1, P], F16, name="ones16")
            nc.vector.memset(ones16, 1.0)
            onesb = const.tile([T, 1], BF16, name="onesb")
            nc.vector.memset(onesb, 1.0)
            onesr = const.tile([1, P], BF16, name="onesr")
            nc.vector.memset(onesr, 1.0)

            def emit_qphase(i, qT4_sb):
                isl = slice(i * T, (i + 1) * T)
                # qn = query_i @ Wq  [T, H] (lhsT for attended^T)
                ps_q = psum.tile([T, H], F32, tag="st", bufs=3, name="ps_q")
                for d in range(DT):
                    nc.tensor.matmul(ps_q[:], qT4_sb[d][:, isl], wq_sb[d][:],
                                     start=(d == 0), stop=(d == DT - 1))
                qn_i = perb.tile([T, H], F16, tag="qn", name="qn_i")
                nc.scalar.activation(qn_i[:], ps_q[:], AFT.Copy)
                # qb = query_i @ (Wq@Wc2) + 1(x)b  [T, H] (rhs for final)
                ps_b = psum.tile([T, H], F32, tag="st", bufs=3, name="ps_b")
                for d in range(DT):
                    nc.tensor.matmul(ps_b[:], qT4_sb[d][:, isl], wqc2_sb[d][:],
                                     start=(d == 0), stop=False)
                nc.tensor.matmul(ps_b[:], ones16[:, :T], bc_sb[:],
                                 start=False, stop=True)
                qb_i = perb.tile([T, H], F16, tag="qb", name="qb_i")
                nc.scalar.activation(qb_i[:], ps_b[:], AFT.Copy)
                return qn_i, qb_i

            pending_final = [None]

            def emit_final(aT_sb, pr_sb, atT, qb_i, i, c):
                def emit(js):
                    for j in js:
                        jsl = slice(j * P, (j + 1) * P)
                        ps_x = psum.tile([P, H], F32, tag="mm512", name="ps_x")
                        # anchor@Wc1 first (no softmax dependency), then the
                        # atT/pr terms so DVE has time to produce them
                        for d in range(DT):
                            nc.tensor.matmul(ps_x[:], aT_sb[d][:, jsl], wc1_sb[d][:],
                                             start=(d == 0), stop=False)
                        nc.tensor.matmul(ps_x[:], atT[:, jsl], qb_i[:],
                                         start=False, stop=False)
                        for h in range(HT):
                            nc.tensor.matmul(ps_x[:], pr_sb[h][:, jsl], wc3_sb[h][:],
                                             start=False, stop=(h == HT - 1))
                        xo = stage.tile([P, H], F16, tag="xo", name="xo")
                        nc.scalar.activation(xo[:], ps_x[:], AFT.Relu)
                        nc.sync.dma_start(
                            out=x[i, c * ACH + j * P: c * ACH + (j + 1) * P, :],
                            in_=xo[:])
                return emit

            for rep in range(reps):
                # ---- per-rep query-side inputs ----
                qT4_sb = []
                for d in range(DT):
                    t = perrep.tile([P, QTW], F16, tag=f"qT{d}", name=f"qT{d}")
                    nc.scalar.dma_start(out=t, in_=qT[d * P:(d + 1) * P, :])
                    qT4_sb.append(t)
                mbT_sb = []
                for i in range(PB):
                    t = perrep.tile([T, 1], F32, tag=f"mbT{i}", name=f"mbT{i}")
                    nc.scalar.dma_start(out=t, in_=mbT[i, :, :])
                    mbT_sb.append(t)
                if rep == 0:
                    # qn/qb weights ride the (otherwise idle) scalar DMA queue
                    # so the sync queue streams mqr + aT without interleave
                    for d in range(DT):
                        nc.scalar.dma_start(out=wq_sb[d],
                                            in_=wq[d * P:(d + 1) * P, :])
                    for d in range(DT):
                        nc.scalar.dma_start(out=wqc2_sb[d],
                                            in_=wqc2[d * P:(d + 1) * P, :])
                # ---- V^T for all 4 batches: [D, 4T] = (WrWq^T) @ q^T ----
                vt_sb = []
                for h in range(DT):
                    ps_v = psum.tile([P, QTW], F32, tag="mm512", name="ps_v")
                    for d in range(DT):
                        nc.tensor.matmul(ps_v[:], mqr_sb[d][:, h * P:(h + 1) * P],
                                         qT4_sb[d][:],
                                         start=(d == 0), stop=(d == DT - 1))
                    t = perrep.tile([P, QTW], F16, tag=f"vt{h}", name=f"vt{h}")
                    nc.vector.tensor_copy(t[:], ps_v[:])
                    vt_sb.append(t)

                for i in range(PB):
                    isl = slice(i * T, (i + 1) * T)
                    qn_i, qb_i = emit_qphase(i, qT4_sb)

                    for c in range(NCH):
                        asl = slice(c * ACH, (c + 1) * ACH)
                        # ---- anchor^T chunk [D, ACH] ----
                        aT_sb = []
                        for d in range(DT):
                            t = chunk.tile([P, ACH], F16, tag=f"aT{d}", name=f"aT{d}")
                            nc.sync.dma_start(out=t, in_=aT[i, d * P:(d + 1) * P, asl])
                            aT_sb.append(t)
                        if rep == 0 and i == 0:
                            # deferred const loads, interleaved between the aT
                            # streams in first-use order
                            if c == 0:
                                for d in range(DT):
                                    nc.sync.dma_start(
                                        out=wc1_sb[d], in_=wc1[d * P:(d + 1) * P, :])
                            else:
                                for d in range(DT):
                                    nc.sync.dma_start(
                                        out=wc3_sb[d], in_=wc3[d * P:(d + 1) * P, :])

                        # ---- S^T [T, ACH] + shifted-exp softmax ----
                        ps_st = psum.tile([T, ACH], F32, tag="st", bufs=3, name="ps_st")
                        for d in range(DT):
                            nc.tensor.matmul(ps_st[:], vt_sb[d][:, isl], aT_sb[d][:],
                                             start=(d == 0), stop=(d == DT - 1))
                        euT = chunk.tile([T, ACH], BF16, tag="euT", name="euT")
                        nc.scalar.activation(euT[:], ps_st[:], AFT.Exp,
                                             bias=mbT_sb[i][:], scale=1.0)
                        # previous chunk's final j=0,1 fills PE while exp runs
                        if pending_final[0] is not None:
                            pending_final[0]((0, 1))
                        ps_sum = psum.tile([1, ACH], F32, tag="st", bufs=3,
                                           name="ps_sum")
                        nc.tensor.matmul(ps_sum[:], onesb[:], euT[:],
                                         start=True, stop=True)
                        rs = small.tile([1, ACH], BF16, tag="rs", name="rs")
                        with nc.allow_low_precision(
                                reason="1/sum needs bf16 range; elementwise"):
                            nc.vector.reciprocal(rs[:], ps_sum[:])
                        # j=2 of the previous final covers the reciprocal
                        if pending_final[0] is not None:
                            pending_final[0]((2,))
                        # broadcast 1/sum across the T partitions via a K=1
                        # matmul (bf16: full rate), normalize straight off psum
                        ps_rsb = psum.tile([T, ACH], F32, tag="st", bufs=3,
                                           name="ps_rsb")
                        nc.tensor.matmul(ps_rsb[:], onesr[:, :T], rs[:],
                                         start=True, stop=True)
                        atT = chunk.tile([T, ACH], F16, tag="atT", name="atT")
                        nc.vector.tensor_mul(atT[:], euT[:], ps_rsb[:])
                        # j=3 covers the normalize multiply
                        if pending_final[0] is not None:
                            pending_final[0]((3,))

                        # ---- attended^T chunk [H, ACH]; product on DVE
                        # straight off the psum ----
                        pr_sb = []
                        for h in range(HT):
                            ps_a = psum.tile([P, ACH], F32, tag="mm512", name="ps_a")
                            nc.tensor.matmul(ps_a[:], qn_i[:, h * P:(h + 1) * P],
                                             atT[:], start=True, stop=True)
                            pr = chunk.tile([P, ACH], F16, tag=f"pr{h}", name=f"pr{h}")
                            nc.vector.tensor_mul(pr[:], aT_sb[h][:], ps_a[:])
                            pr_sb.append(pr)

                        pending_final[0] = emit_final(aT_sb, pr_sb, atT, qb_i, i, c)
            pending_final[0]((0, 1, 2, 3))
    nc.compile()
    return nc


def _prep(anchor_feats, query_embs, query_mask, W_region, W_query, W_combine, b_combine):
    """Host-side shard + layout prep. Returns the 8 per-core input maps."""
    f32, f16 = np.float32, np.float16
    a = np.asarray(anchor_feats, dtype=f32).reshape(NCORES, PB, A, D)
    aTv = np.ascontiguousarray(a.transpose(0, 1, 3, 2)).astype(f16)
    q = np.asarray(query_embs, dtype=f32).reshape(NCORES, PB * T, D)
    qTv = np.ascontiguousarray(q.transpose(0, 2, 1)).astype(f16)  # [NC, D, PB*T]
    mbTv = np.where(np.asarray(query_mask).reshape(NCORES, PB, T, 1) > 0,
                    f32(0), f32(-30000)).astype(f32) - f32(SHIFT)
    Wq = np.asarray(W_query, dtype=f32)
    Wr = np.asarray(W_region, dtype=f32)
    Wc = np.asarray(W_combine, dtype=f32)
    mqrv = np.ascontiguousarray(Wq @ Wr.T).astype(f16)             # [Dq, Dr]
    wqc2v = np.ascontiguousarray(Wq @ Wc[H:2 * H, :]).astype(f16)  # [D, H]
    wqv = Wq.astype(f16)
    wc1v = np.ascontiguousarray(Wc[:H, :]).astype(f16)
    wc3v = np.ascontiguousarray(Wc[2 * H:, :]).astype(f16)
    bcv = np.asarray(b_combine, dtype=f32).reshape(1, H).astype(f16)
    return [
        {"aT": aTv[cid], "qT": qTv[cid], "mbT": mbTv[cid], "wq": wqv,
         "mqr": mqrv, "wqc2": wqc2v, "wc1": wc1v, "wc3": wc3v, "bc": bcv}
        for cid in range(NCORES)
    ]


def kernel(anchor_feats, query_embs, query_mask,
           W_region, W_query, W_combine, b_combine):
    if "nc" not in _CACHE:
        _CACHE["nc"] = build()
    nc = _CACHE["nc"]
    in_maps = _prep(anchor_feats, query_embs, query_mask,
                    W_region, W_query, W_combine, b_combine)
    res = run_bass_kernel_spmd(nc, in_maps, core_ids=list(range(NCORES)))
    out = np.empty((B, A, H), dtype=np.float32)
    for cid in range(NCORES):
        out[cid * PB:(cid + 1) * PB] = res.results[cid]["x"].astype(np.float32)
    return out


# revision 3
# speedup vs baseline: 29.6761x; 29.6761x over previous
"""CrossModalAttentionScorer Trainium2 kernel (Bass/Tile, 8 NeuronCores), v4.

Reference computation (per batch b):
    R = anchor @ W_region            [A, H]
    Q = query  @ W_query             [T, H]
    S = R @ Q.T  (masked over T)     [A, T]
    P = softmax(S, axis=T)
    att = P @ Q                      [A, H]
    out = relu(concat([anchor, att, anchor*att]) @ W_combine + b)   [A, H]

Shapes: B=32, A=1024, T=64, D=H=512. Data-parallel over B across 8 cores.

Math restructuring (see v2/v3 history):
  * R only feeds S, so S = anchor @ (Wr Wq^T) @ q^T with M = Wq Wr^T applied
    to the tiny query side; att@Wc2 = P @ (q @ (Wq Wc2)); the combine bias
    rides the qb projection since softmax rows sum to 1.
  * S^T [T, ACH] layout: 4 N=512 matmuls/chunk, mask+shift folded into the
    exp bias (constant-shift softmax, no row max -- logits are N(0, 22.6), so
    exp(s-40) is range-safe in bf16/f32 by ~40 sigma), row sums via a
    ones-matmul, 1/sum broadcast via a K=1 matmul, one DVE multiply yields
    normalized attn^T in fp16 directly (no PE transposes at all).
  * fp16 operands everywhere (fp32 psum), bf16 for the exp/softmax internals.

v4: DMA batching. Every dma_start costs ~650ns on its issuing sequencer and
625ns on the single shared HWDGE dispatcher, plus a 900ns completion-sem
delay -- at v3's 93 DMAs/rep that rivals the PE's 89us of matmul work. The
host now packs inputs so each logical stream is ONE dma_start with multi-KB
contiguous runs:
  * anchor: [PB, 128, 4*A] (d-tiles side by side per partition; 8KB rows) --
    1 DMA per batch instead of 8.
  * weights: two packs (mqr | wq | wqc2) and (wc1 | wc3) of [128, k*512].
  * q^T: [128, 4*256]; masks: [T, PB] (bias column per batch).
  * output: dram laid out [PB, NCH, 128, AT_CH, H] so a whole chunk's relu
    output [128, 4*H] stores as one DMA with 4KB runs; host untiles.
"""
import numpy as np

import concourse.bacc as bacc
import concourse.tile as tile
import concourse.mybir as mybir
from concourse.bass_utils import run_bass_kernel_spmd

B, A, T, D, H = 32, 1024, 64, 512, 512
NCORES = 8
PB = B // NCORES          # batches per core = 4
P = 128                   # partitions
DT = D // P               # 4 d-tiles
HT = H // P               # 4 h-tiles
ACH = 512                 # a-chunk (moving-dim) size
NCH = A // ACH            # 2 chunks per batch
AT_CH = ACH // P          # 4 a-tiles per chunk
QTW = PB * T              # packed q width = 256
SHIFT = 40.0              # constant softmax shift

F32 = mybir.dt.float32
F16 = mybir.dt.float16
BF16 = mybir.dt.bfloat16
AFT = mybir.ActivationFunctionType

_CACHE = {}


def build(reps: int = 1):
    """Build the per-core Bass module (4 batches of the problem).

    reps>1 repeats the whole computation in one NEFF for timing-by-slope."""
    nc = bacc.Bacc(None, target_bir_lowering=False, debug=False)

    aT = nc.dram_tensor("aT", [PB, P, DT * A], F16, kind="ExternalInput")
    qT = nc.dram_tensor("qT", [P, DT * QTW], F16, kind="ExternalInput")
    # mask bias at rows (i%2)*T for pair i//2: [2T, PB//2]
    mbT = nc.dram_tensor("mbT", [P, PB // 2], F32, kind="ExternalInput")
    wpk1 = nc.dram_tensor("wpk1", [P, 12 * H], F16, kind="ExternalInput")
    wpk2 = nc.dram_tensor("wpk2", [P, 8 * H], F16, kind="ExternalInput")
    bc = nc.dram_tensor("bc", [1, H], F16, kind="ExternalInput")
    x = nc.dram_tensor("x", [PB, NCH, P, AT_CH * H], F16, kind="ExternalOutput")

    with tile.TileContext(nc) as tc:
        with (
            tc.tile_pool(name="const", bufs=1) as const,
            tc.tile_pool(name="perrep", bufs=2) as perrep,
            tc.tile_pool(name="perb", bufs=2) as perb,
            tc.tile_pool(name="chunk", bufs=2) as chunk,
            tc.tile_pool(name="small", bufs=4) as small,
            tc.tile_pool(name="stage", bufs=2) as stage,
            tc.tile_pool(name="psum", bufs=4, space="PSUM") as psum,
        ):
            # ---- constants: two packed tiles, sliced into views ----
            w1_sb = const.tile([P, 12 * H], F16, name="w1_sb")
            nc.sync.dma_start(out=w1_sb, in_=wpk1[:, :])
            w2_sb = const.tile([P, 8 * H], F16, name="w2_sb")
            mqr_sb = [w1_sb[:, (0 + d) * H:(1 + d) * H] for d in range(DT)]
            wq_sb = [w1_sb[:, (4 + d) * H:(5 + d) * H] for d in range(DT)]
            wqc2_sb = [w1_sb[:, (8 + d) * H:(9 + d) * H] for d in range(DT)]
            wc1_sb = [w2_sb[:, (0 + d) * H:(1 + d) * H] for d in range(DT)]
            wc3_sb = [w2_sb[:, (4 + d) * H:(5 + d) * H] for d in range(DT)]
            bc_sb = const.tile([1, H], F16, name="bc_sb")
            nc.scalar.dma_start(out=bc_sb, in_=bc[:, :])
            ones16 = const.tile([1, P], F16, name="ones16")
            nc.vector.memset(ones16, 1.0)
            onesb = const.tile([P, 1], BF16, name="onesb")
            nc.vector.memset(onesb, 1.0)
            onesr = const.tile([1, P], BF16, name="onesr")
            nc.vector.memset(onesr, 1.0)

            def emit_qphase(pair, qT4_sb):
                # two batches (2*pair, 2*pair+1) share one [2T, H] projection;
                # batch i lives at partition rows (i%2)*T
                psl = slice(pair * 2 * T, (pair + 1) * 2 * T)
                # qn = query @ Wq  [2T, H] (lhsT for attended^T)
                ps_q = psum.tile([P, H], F32, tag="st", bufs=3, name="ps_q")
                for d in range(DT):
                    nc.tensor.matmul(ps_q[:], qT4_sb[d][:, psl], wq_sb[d],
                                     start=(d == 0), stop=(d == DT - 1))
                qn_p = perb.tile([P, H], F16, tag="qn", name="qn_p")
                nc.scalar.activation(qn_p[:], ps_q[:], AFT.Copy)
                # qb = query @ (Wq@Wc2) + 1(x)b  [2T, H] (rhs for final)
                ps_b = psum.tile([P, H], F32, tag="st", bufs=3, name="ps_b")
                for d in range(DT):
                    nc.tensor.matmul(ps_b[:], qT4_sb[d][:, psl], wqc2_sb[d],
                                     start=(d == 0), stop=False)
                nc.tensor.matmul(ps_b[:], ones16[:], bc_sb[:],
                                 start=False, stop=True)
                qb_p = perb.tile([P, H], F16, tag="qb", name="qb_p")
                nc.scalar.activation(qb_p[:], ps_b[:], AFT.Copy)
                return qn_p, qb_p

            pending_final = [None]

            def emit_final(aT_c, pr_sb, atT, qb_p, r0, i, c):
                xo = stage.tile([P, AT_CH * H], F16, tag="xo", name="xo")
                rsl = slice(r0, r0 + T)

                def emit(js):
                    for j in js:
                        jsl = slice(j * P, (j + 1) * P)
                        ps_x = psum.tile([P, H], F32, tag="mm512", name="ps_x")
                        # anchor@Wc1 first (no softmax dependency), then the
                        # atT/pr terms so DVE has time to produce them
                        for d in range(DT):
                            nc.tensor.matmul(ps_x[:], aT_c[d][:, jsl], wc1_sb[d],
                                             start=(d == 0), stop=False)
                        nc.tensor.matmul(ps_x[:], atT[rsl, jsl], qb_p[rsl, :],
                                         start=False, stop=False)
                        for h in range(HT):
                            nc.tensor.matmul(ps_x[:], pr_sb[h][:, jsl], wc3_sb[h],
                                             start=False, stop=(h == HT - 1))
                        nc.scalar.activation(xo[:, j * H:(j + 1) * H], ps_x[:],
                                             AFT.Relu)
                        if j == AT_CH - 1:
                            nc.sync.dma_start(out=x[i, c], in_=xo[:])
                return emit

            for rep in range(reps):
                # ---- per-rep query-side inputs (one DMA each) ----
                qTall = perrep.tile([P, DT * QTW], F16, tag="qTall", name="qTall")
                nc.scalar.dma_start(out=qTall, in_=qT[:, :])
                qT4_sb = [qTall[:, d * QTW:(d + 1) * QTW] for d in range(DT)]
                mbT4 = perrep.tile([P, PB // 2], F32, tag="mbT4", name="mbT4")
                nc.scalar.dma_start(out=mbT4, in_=mbT[:, :])
                # ---- V^T for all 4 batches: [D, 4T] = (WrWq^T) @ q^T ----
                vt_sb = []
                for h in range(DT):
                    ps_v = psum.tile([P, QTW], F32, tag="mm512", name="ps_v")
                    for d in range(DT):
                        nc.tensor.matmul(ps_v[:], mqr_sb[d][:, h * P:(h + 1) * P],
                                         qT4_sb[d],
                                         start=(d == 0), stop=(d == DT - 1))
                    t = perrep.tile([P, QTW], F16, tag=f"vt{h}", name=f"vt{h}")
                    nc.vector.tensor_copy(t[:], ps_v[:])
                    vt_sb.append(t)

                qn_p = qb_p = None
                for i in range(PB):
                    isl = slice(i * T, (i + 1) * T)
                    r0 = (i % 2) * T          # partition row base for batch i
                    rsl = slice(r0, r0 + T)
                    # whole-batch anchor^T in one DMA: [P, 4*A], d-major blocks
                    aTall = perb.tile([P, DT * A], F16, tag="aTall", name="aTall")
                    nc.sync.dma_start(out=aTall, in_=aT[i])
                    if rep == 0 and i == 0:
                        # wc1/wc3 ride behind the first anchor load; first use
                        # is the first pending_final, a chunk later
                        nc.sync.dma_start(out=w2_sb, in_=wpk2[:, :])
                    if i % 2 == 0:
                        qn_p, qb_p = emit_qphase(i // 2, qT4_sb)

                    for c in range(NCH):
                        aT_c = [aTall[:, d * A + c * ACH: d * A + (c + 1) * ACH]
                                for d in range(DT)]
                        # ---- S^T [T, ACH] + shifted-exp softmax, on partition
                        # rows r0:r0+T so the pair-packed qn/qb slices align ----
                        ps_st = psum.tile([P, ACH], F32, tag="st", bufs=3,
                                          name="ps_st")
                        for d in range(DT):
                            nc.tensor.matmul(ps_st[rsl, :], vt_sb[d][:, isl], aT_c[d],
                                             start=(d == 0), stop=(d == DT - 1))
                        euT = chunk.tile([P, ACH], BF16, tag="euT", name="euT")
                        nc.scalar.activation(euT[rsl, :], ps_st[rsl, :], AFT.Exp,
                                             bias=mbT4[rsl, i // 2:i // 2 + 1],
                                             scale=1.0)
                        # previous chunk's final j=0,1 fills PE while exp runs
                        if pending_final[0] is not None:
                            pending_final[0]((0, 1))
                        ps_sum = psum.tile([1, ACH], F32, tag="st", bufs=3,
                                           name="ps_sum")
                        nc.tensor.matmul(ps_sum[:], onesb[rsl, :], euT[rsl, :],
                                         start=True, stop=True)
                        rs = small.tile([1, ACH], BF16, tag="rs", name="rs")
                        with nc.allow_low_precision(
                                reason="1/sum needs bf16 range; elementwise"):
                            nc.vector.reciprocal(rs[:], ps_sum[:])
                        # j=2 of the previous final covers the reciprocal
                        if pending_final[0] is not None:
                            pending_final[0]((2,))
                        # broadcast 1/sum across the T partitions via a K=1
                        # matmul (bf16: full rate), normalize straight off psum
                        ps_rsb = psum.tile([P, ACH], F32, tag="st", bufs=3,
                                           name="ps_rsb")
                        nc.tensor.matmul(ps_rsb[rsl, :], onesr[:, :T], rs[:],
                                         start=True, stop=True)
                        atT = chunk.tile([P, ACH], F16, tag="atT", name="atT")
                        nc.vector.tensor_mul(atT[rsl, :], euT[rsl, :], ps_rsb[rsl, :])
                        # j=3 covers the normalize multiply
                        if pending_final[0] is not None:
                            pending_final[0]((3,))

                        # ---- attended^T chunk [H, ACH]; product on DVE
                        # straight off the psum ----
                        pr_sb = []
                        for h in range(HT):
                            ps_a = psum.tile([P, ACH], F32, tag="mm512", name="ps_a")
                            nc.tensor.matmul(ps_a[:], qn_p[rsl, h * P:(h + 1) * P],
                                             atT[rsl, :], start=True, stop=True)
                            pr = chunk.tile([P, ACH], F16, tag=f"pr{h}", name=f"pr{h}")
                            nc.vector.tensor_mul(pr[:], aT_c[h], ps_a[:])
                            pr_sb.append(pr)

                        pending_final[0] = emit_final(aT_c, pr_sb, atT, qb_p,
                                                      r0, i, c)
            pending_final[0]((0, 1, 2, 3))
    nc.compile()
    return nc


def _prep(anchor_feats, query_embs, query_mask, W_region, W_query, W_combine, b_combine):
    """Host-side shard + layout prep. Returns the 8 per-core input maps."""
    f32, f16 = np.float32, np.float16

    def dtile(w):  # [512, N] -> [4, 128, N] d-tiles
        return np.asarray(w, dtype=f32).reshape(DT, P, -1)

    a = np.asarray(anchor_feats, dtype=f32).reshape(NCORES, PB, A, DT, P)
    # aT[cid, i, p, d*A + a] = anchor[., a, d*128+p]
    aTv = np.ascontiguousarray(a.transpose(0, 1, 4, 3, 2)
                               ).reshape(NCORES, PB, P, DT * A).astype(f16)
    q = np.asarray(query_embs, dtype=f32).reshape(NCORES, PB * T, DT, P)
    # qT[cid, p, d*QTW + t'] = query[., t', d*128+p]
    qTv = np.ascontiguousarray(q.transpose(0, 3, 2, 1)
                               ).reshape(NCORES, P, DT * QTW).astype(f16)
    # mask bias: batch i at rows (i%2)*T of column i//2: [2T, PB//2]
    mb = (np.where(np.asarray(query_mask).reshape(NCORES, PB // 2, 2, T) > 0,
                   f32(0), f32(-30000)) - f32(SHIFT))
    mbTv = np.ascontiguousarray(mb.transpose(0, 2, 3, 1)
                                ).reshape(NCORES, P, PB // 2).astype(f32)
    Wq = np.asarray(W_query, dtype=f32)
    Wr = np.asarray(W_region, dtype=f32)
    Wc = np.asarray(W_combine, dtype=f32)
    wpk1v = np.concatenate(
        [*dtile(Wq @ Wr.T), *dtile(Wq), *dtile(Wq @ Wc[H:2 * H, :])],
        axis=1).astype(f16)                                     # [128, 12*H]
    wpk2v = np.concatenate(
        [*dtile(Wc[:H, :]), *dtile(Wc[2 * H:, :])], axis=1).astype(f16)
    bcv = np.asarray(b_combine, dtype=f32).reshape(1, H).astype(f16)
    return [
        {"aT": aTv[cid], "qT": qTv[cid], "mbT": mbTv[cid],
         "wpk1": np.ascontiguousarray(wpk1v),
         "wpk2": np.ascontiguousarray(wpk2v), "bc": bcv}
        for cid in range(NCORES)
    ]


def kernel(anchor_feats, query_embs, query_mask,
           W_region, W_query, W_combine, b_combine):
    if "nc" not in _CACHE:
        _CACHE["nc"] = build()
    nc = _CACHE["nc"]
    in_maps = _prep(anchor_feats, query_embs, query_mask,
                    W_region, W_query, W_combine, b_combine)
    res = run_bass_kernel_spmd(nc, in_maps, core_ids=list(range(NCORES)))
    out = np.empty((B, A, H), dtype=np.float32)
    for cid in range(NCORES):
        # x dram is [PB, NCH, P, AT_CH, H]-tiled; untile to [PB, A, H]
        xt = res.results[cid]["x"].reshape(PB, NCH, P, AT_CH, H)
        xt = xt.transpose(0, 1, 3, 2, 4).reshape(PB, A, H)
        out[cid * PB:(cid + 1) * PB] = xt.astype(np.float32)
    return out
